# revision 1
# baseline (speedup 1.0000x reference)
"""DimGraphConv (GNN mean-aggregation message passing) on 8 Trainium2 cores.

Math (per reference):
    out = x @ W_self.T + segment_mean(x[row], col) @ W_neigh.T + bias

Because the neighbor transform is linear, we aggregate raw features first
(segment-sum + in-degree on device), then apply one small GEMM per 128-node
slot:  out^T = W_neigh @ (agg/deg)^T + W_self @ x^T + bias.

Sharding ("scatter to node owners" variant of the hint): edges are
partitioned on the host by TARGET owner -- core c owns nodes
[c*12500, (c+1)*12500) -- so no cross-core collective is needed. x is
replicated; each core dma_gathers its edges' source rows (256B each) from
HBM and accumulates them into an SBUF-resident accumulator with the SDMA
CCE scatter-add (dma_scatter_add, parity-split SBUF destination mode,
tokens_per_rank=128). A second elem16 scatter of ones accumulates the
in-degree.

Hardware constraints handled on the host (pure edge reordering/padding --
the sums themselves all happen on device):
  * gather indices are int16 -> edges are grouped by source bank
    (4 banks x 25000 rows); each gather instruction reads one bank.
  * CCE read-modify-write races when two tokens in the SAME scatter
    instruction hit the same cell -> within each chunk all targets are
    distinct. Edges of one (core, bank) are dealt into NCHB chunks so that
    equal-target edges land in different chunks (offset round-robin).
  * idx tiles must hold the [16, n/16]-wrapped pattern replicated across
    all 8 16-partition groups (different Q7 cores read different copies).
  * pad tokens gather row 0 and scatter into a dead cell (node id 12500),
    so every token is valid and the SPMD program is identical on all cores.
"""
import sys

sys.path.insert(0, "/opt/trn_rl_repo")

import numpy as np


# ---------------------------------------------------------------- config
class _Cfg:
    N = 100000          # nodes
    D = 64              # feature dim (256B rows = dma_gather elem)
    NCORE = 8
    W = 12500           # target window per core
    NBANK = 4           # source banks (int16 gather index limit)
    BANK = 25000        # rows per source bank
    M = 2816            # tokens per chunk (22*128)
    NCHB = 20           # chunks per (core, bank); max per-bucket target
                        # multiplicity must stay < NCHB (Poisson(4) here)
    EW = 68             # scatter payload elems: 64 features + degree + pad
    REPEAT = 1          # timing-only: re-run the phase-1 loop this many times
    GRP = 7             # phase-2 slots per output DMA group

    @property
    def BANK_CAP(self):
        return self.M * self.NCHB

    @property
    def NCH(self):
        return self.NBANK * self.NCHB

    @property
    def TOK(self):
        return self.NBANK * self.BANK_CAP

    @property
    def NSLOT(self):
        # slots of 128 target nodes; last slot also holds the dead pad cell
        return self.W // 128 + 1

    @property
    def NGRP(self):
        return (self.NSLOT + 1) // 2

    @property
    def WPAD(self):
        return self.NSLOT * 128

    @property
    def CPC(self):
        return self.M // 16


CFG = _Cfg()


def _build(cfg):
    """Build + compile the SPMD Bass program. Returns the Bacc instance."""
    import concourse.tile as tile
    from concourse import bacc, mybir
    from concourse.masks import make_identity

    P = 128
    D = cfg.D
    MS = cfg.M // P              # gather tile free rows
    f32 = mybir.dt.float32

    nc = bacc.Bacc(None, target_bir_lowering=False, debug=False)
    x_d = nc.dram_tensor("x", [cfg.N, D], f32, kind="ExternalInput")
    xT_d = nc.dram_tensor("xT", [D, cfg.WPAD], f32, kind="ExternalInput")
    gidx_d = nc.dram_tensor("gidx", [128, cfg.TOK // 16], mybir.dt.int16,
                            kind="ExternalInput")
    sidx_d = nc.dram_tensor("sidx", [128, cfg.TOK // 16], mybir.dt.int16,
                            kind="ExternalInput")
    wnT_d = nc.dram_tensor("wnT", [D, D], f32, kind="ExternalInput")
    wsT_d = nc.dram_tensor("wsT", [D, D], f32, kind="ExternalInput")
    bias_d = nc.dram_tensor("bias", [D, 1], f32, kind="ExternalInput")
    outT_d = nc.dram_tensor("outT", [D, cfg.WPAD], f32, kind="ExternalOutput")

    with tile.TileContext(nc) as tc:
        with (
            tc.tile_pool(name="acc", bufs=1) as accp,
            tc.tile_pool(name="io", bufs=4) as iop,
            tc.tile_pool(name="p2", bufs=3) as p2p,
            tc.tile_pool(name="psum", bufs=2, space="PSUM") as psp,
        ):
            gidx_t = accp.tile([128, cfg.TOK // 16], mybir.dt.int16)
            sidx_t = accp.tile([128, cfg.TOK // 16], mybir.dt.int16)
            nc.sync.dma_start(gidx_t[:], gidx_d[:])
            nc.sync.dma_start(sidx_t[:], sidx_d[:])

            EW = cfg.EW
            own_t = accp.tile([P, cfg.NGRP * EW], f32)
            peer_t = accp.tile([P, cfg.NGRP * EW], f32)
            nc.vector.memset(own_t[:], 0.0)
            nc.vector.memset(peer_t[:], 0.0)

            # ---- phase 1: gather sources + scatter-add into SBUF acc
            for _rep in range(cfg.REPEAT):
              for ch in range(cfg.NCH):
                  bank = ch // cfg.NCHB
                  sl = slice(ch * cfg.CPC, (ch + 1) * cfg.CPC)
                  gath_t = iop.tile([P, MS * D], f32)
                  nc.gpsimd.dma_gather(
                      out_ap=gath_t[:].rearrange("p (m e) -> p m e", e=D),
                      in_ap=x_d[bank * cfg.BANK:(bank + 1) * cfg.BANK, :],
                      idxs_ap=gidx_t[:, sl],
                      num_idxs=cfg.M,
                      num_idxs_reg=cfg.M,
                      elem_size=D,
                      single_packet=False,
                  )
                  # widen each token row to 68 f32: cols 0..63 features,
                  # col 64 = 1.0 (degree), cols 65..67 unused filler
                  aug_t = iop.tile([P, MS * EW], f32)
                  aug3 = aug_t[:].rearrange("p (m e) -> p m e", e=EW)
                  nc.vector.memset(aug3[:, :, D:EW], 1.0)
                  nc.vector.tensor_copy(
                      aug3[:, :, 0:D],
                      gath_t[:].rearrange("p (m e) -> p m e", e=D))
                  nc.gpsimd.dma_scatter_add(
                      out_ap=own_t[:],
                      in_ap=aug3,
                      idxs_ap=sidx_t[:, sl],
                      num_idxs=cfg.M,
                      num_idxs_reg=cfg.M,
                      elem_size=EW,
                      out_ap_other=peer_t[:],
                      parity_reg=0,
                      sbuf_tokens_per_rank=128,
                      single_packet=False,
                  )

            # ---- phase 2: mean, two GEMMs, bias; output transposed
            ident_t = p2p.tile([P, P], f32)
            make_identity(nc, ident_t[:])
            wnT_t = p2p.tile([D, D], f32)
            wsT_t = p2p.tile([D, D], f32)
            bias_t = p2p.tile([D, 1], f32)
            nc.sync.dma_start(wnT_t[:], wnT_d[:])
            nc.sync.dma_start(wsT_t[:], wsT_d[:])
            nc.sync.dma_start(bias_t[:], bias_d[:])

            ngroups_out = cfg.NSLOT // cfg.GRP
            assert ngroups_out * cfg.GRP == cfg.NSLOT
            gw = cfg.GRP * P
            for og in range(ngroups_out):
                xTg_t = p2p.tile([D, gw], f32)
                outg_t = p2p.tile([D, gw], f32)
                nc.sync.dma_start(xTg_t[:], xT_d[:, og * gw:(og + 1) * gw])
                for k in range(cfg.GRP):
                    s = og * cfg.GRP + k
                    g = s >> 1
                    accb = own_t if (s & 1) == 0 else peer_t
                    acc_sl = accb[:, g * EW:g * EW + D]
                    deg_sl = accb[:, g * EW + D:g * EW + D + 1]
                    degc_t = p2p.tile([P, 1], f32)
                    recip_t = p2p.tile([P, 1], f32)
                    mean_t = p2p.tile([P, D], f32)
                    nc.vector.tensor_scalar_max(degc_t[:], deg_sl, 1.0)
                    nc.vector.reciprocal(recip_t[:], degc_t[:])
                    nc.vector.tensor_scalar_mul(mean_t[:], acc_sl,
                                                recip_t[:, 0:1])
                    psT_t = psp.tile([D, P], f32)
                    nc.tensor.transpose(psT_t[:], mean_t[:], ident_t[:])
                    meanT_t = p2p.tile([D, P], f32)
                    nc.vector.tensor_copy(meanT_t[:], psT_t[:])
                    po_t = psp.tile([D, P], f32)
                    nc.tensor.matmul(po_t[:], lhsT=wnT_t[:], rhs=meanT_t[:],
                                     start=True, stop=False)
                    nc.tensor.matmul(po_t[:], lhsT=wsT_t[:],
                                     rhs=xTg_t[:, k * P:(k + 1) * P],
                                     start=False, stop=True)
                    nc.vector.tensor_scalar_add(outg_t[:, k * P:(k + 1) * P],
                                                po_t[:], bias_t[:, 0:1])
                nc.sync.dma_start(outT_d[:, og * gw:(og + 1) * gw], outg_t[:])

    nc.compile()
    return nc


def _pack_bucket(t_local, lo, nchb, m):
    """Deal one (core, bank) bucket's edges into nchb chunks of capacity m
    such that within a chunk all targets are distinct. Returns (chunk_id,
    pos_in_chunk) per edge, aligned with the input order.

    Equal-target occurrences get chunk (occ_idx + hash(t)) % nchb, which is
    injective per target as long as multiplicity <= nchb. A few hash seeds
    are tried if a chunk overflows its capacity."""
    n = len(t_local)
    if n == 0:
        return np.zeros(0, np.int64), np.zeros(0, np.int64)
    order = np.argsort(t_local, kind="stable")
    ts = t_local[order].astype(np.int64)
    first = np.r_[True, ts[1:] != ts[:-1]]
    runstart = np.maximum.accumulate(np.where(first, np.arange(n), 0))
    occ = np.arange(n) - runstart
    kmax = int(occ.max())
    if kmax >= nchb:
        raise RuntimeError(
            f"target multiplicity {kmax + 1} exceeds chunk count {nchb}")
    for seed in range(16):
        h = ((ts * 2654435761 + seed * 97) % nchb).astype(np.int64)
        chunk_s = (occ + h) % nchb
        sizes = np.bincount(chunk_s, minlength=nchb)
        if sizes.max() <= m:
            break
    else:
        raise RuntimeError(f"chunk overflow: max {sizes.max()} > {m}")
    ord2 = np.argsort(chunk_s, kind="stable")
    starts = np.cumsum(np.r_[0, sizes[:-1]])
    pos_sorted = np.arange(n) - np.repeat(starts, sizes)
    pos_s = np.empty(n, np.int64)
    pos_s[ord2] = pos_sorted
    chunk = np.empty(n, np.int64)
    pos = np.empty(n, np.int64)
    chunk[order] = chunk_s
    pos[order] = pos_s
    return chunk, pos


def _prep_inputs(cfg, x, edge_index, W_self, W_neigh, bias):
    """Host-side sharding: partition edges by target owner, group by source
    bank, deal into duplicate-free chunks, pad, int16-encode, 16-partition
    wrap, replicate across the 8 Q7 groups."""
    x = np.ascontiguousarray(np.asarray(x, dtype=np.float32))
    ei = np.asarray(edge_index)
    row = ei[0].astype(np.int64)
    col = ei[1].astype(np.int64)
    wnT = np.ascontiguousarray(np.asarray(W_neigh, np.float32).T)
    wsT = np.ascontiguousarray(np.asarray(W_self, np.float32).T)
    bias_c = np.ascontiguousarray(
        np.asarray(bias, np.float32).reshape(cfg.D, 1))

    owner = col // cfg.W
    in_maps = []
    for c in range(cfg.NCORE):
        msk = owner == c
        r = row[msk]
        t = (col[msk] - c * cfg.W).astype(np.int64)
        b = r // cfg.BANK
        lo = r % cfg.BANK
        gbuf = np.zeros(cfg.TOK, np.int16)
        sbuf = np.full(cfg.TOK, cfg.W, np.int16)   # pads -> dead cell
        for bk in range(cfg.NBANK):
            sel = b == bk
            k = int(sel.sum())
            if k > cfg.BANK_CAP:
                raise RuntimeError(
                    f"bucket overflow: core {c} bank {bk} has {k} edges "
                    f"(capacity {cfg.BANK_CAP})")
            chunk, pos = _pack_bucket(t[sel], lo[sel], cfg.NCHB, cfg.M)
            slot = bk * cfg.BANK_CAP + chunk * cfg.M + pos
            gbuf[slot] = lo[sel].astype(np.int16)
            sbuf[slot] = t[sel].astype(np.int16)
        xw = np.zeros((cfg.D, cfg.WPAD), np.float32)
        xw[:, :cfg.W] = x[c * cfg.W:(c + 1) * cfg.W].T
        in_maps.append({
            "x": x,
            "xT": xw,
            "gidx": np.tile(np.ascontiguousarray(
                gbuf.reshape(-1, 16).T), (8, 1)),
            "sidx": np.tile(np.ascontiguousarray(
                sbuf.reshape(-1, 16).T), (8, 1)),
            "wnT": wnT,
            "wsT": wsT,
            "bias": bias_c,
        })
    return in_maps


_CACHED_NC = None


def _get_nc():
    global _CACHED_NC
    if _CACHED_NC is None:
        _CACHED_NC = _build(CFG)
    return _CACHED_NC


def kernel(x, edge_index, W_self, W_neigh, bias, _trace=False, _trace_kwargs=None):
    from concourse.bass_utils import run_bass_kernel_spmd

    cfg = CFG
    nc = _get_nc()
    in_maps = _prep_inputs(cfg, x, edge_index, W_self, W_neigh, bias)
    kw = {}
    if _trace:
        kw["trace"] = True
        if _trace_kwargs:
            kw.update(_trace_kwargs)
    res = run_bass_kernel_spmd(nc, in_maps, list(range(cfg.NCORE)), **kw)
    out = np.concatenate(
        [res.results[c]["outT"][:, :cfg.W].T for c in range(cfg.NCORE)], axis=0)
    out = np.ascontiguousarray(out, dtype=np.float32)
    if _trace:
        return out, res
    return out

